# revision 17
# baseline (speedup 1.0000x reference)
"""Single-head attention on 8 Trainium2 NeuronCores.

Problem: x[4,4096,1024] @ {Wq,Wk,Wv}[1024,64] -> scaled-dot-product
attention per batch -> out[4,4096,64].

Sharding: core c handles batch b=c//2, query half h=c%2 (2048 queries),
with K/V over the full 4096-row sequence of its batch. No collectives:
each core receives its batch's x pre-transposed, pre-permuted (own query
half in columns 0:2048) and pre-chunked into contiguous 512-column
pieces so every load DMA is a plain linear copy.

Dataflow per core (all matmul operands contraction-on-partitions):
  xT pieces [128, 8e, 512] bf16
  QT[128,2048] = [Wq|Wq]^T xT(own half)   (lo+hi copies for row-packing)
  VT/KT[128,4096] = [Wv|Wk]^T xT          (VT rows 0:64, KT rows 64:128)
  KT_lo = KT moved to partitions 0:64 via SBUF->SBUF DMA
  V'[s-tile] = PE-transpose of VT (+ DVE copy), ones column appended
  scoresT[s,t]: K=64 matmuls; even s-tile on PE rows 0:63 (ktlo), odd
       s-tile on rows 64:127 (vk_sb) -> the two run concurrently.
  PT = exp(scoresT/8)  (ScalarE, scale fused; scores bounded ~|8| so no
       running-max is needed for fp32 softmax)
  outT[65,t] += V'[s]^T PT[s]  accumulated over all 32 s-tiles in PSUM;
       row 64 is the softmax denominator (ones column).

Scheduling: the kernel is ScalarE/PE co-bound (64 exp tiles ~71us of
ScalarE; ~75us of PE). The Tile scheduler is a greedy priority-driven
list scheduler, so the attention stream (scores -> exp -> AV -> flush)
is emitted under high_priority: whenever its next instruction is ready
it wins the engine, and the projection matmuls + V' transposes back-fill
PE slack with dependencies as natural backpressure (a chunk's
projections run exactly when the attention stream stalls wanting them).
A post-pass drops LDWEIGHTS instructions that reload the stationary
operand the PE already holds (AV half pairs, warm-up). Host divides
rows 0:64 by row 64 and transposes back.
"""

import numpy as np

B, T, E, D = 4, 4096, 1024, 64
HALF = T // 2  # queries per core
NCORES = 8

_compiled = {}


def _patch_tile_drain():
    """This walrus build accepts only one sem-wait on the TileContext exit
    drain; spread the waits across preceding nofuse NOPs instead."""
    import concourse.tile as tile
    import concourse.mybir as mybir
    from concourse.tile import ScopedClock

    if getattr(tile.TileContext, "_drain_patch_installed", False):
        return

    def _drain_and_barrier(self, tick_clock, wait_clock):
        nops = [
            self.nc.sync.nop(nofuse=True, hint=f"drain_wait_{i}") for i in range(26)
        ]
        drain_inst = self.nc.sync.drain()
        wait_clock.add_sem_waits(
            drain_inst.ins, ScopedClock({None: tick_clock.global_clock})
        )
        si = drain_inst.ins.sync_info
        if si is not None and len(si.on_wait) > 1:
            waits = list(si.on_wait)
            assert len(waits) - 1 <= len(nops), f"{len(waits)} drain waits"
            si.on_wait = [waits[-1]]
            for w, nop in zip(waits[:-1], nops):
                nop.ins.sync_info = mybir.SyncInfo(on_wait=[w], on_update=[])

        self.nc.all_engine_barrier()
        assert self.sems is not None
        popped = self.nc._tile_sem_poison_stack.pop()
        assert popped is self._sem_poison
        self.nc.clear_and_free_semaphores(list(self.sems.allocated().values()))
        self.nc.all_engine_barrier()

    tile.TileContext._drain_and_barrier = _drain_and_barrier
    tile.TileContext._drain_patch_installed = True


def _dedupe_ldweights(nc):
    """Drop InstLdweights that reload the exact stationary operand the PE
    array already holds (only matmuls in between on the PE queue). The
    matmul still carries its weights operand; the array state is valid."""
    import concourse.mybir as mybir

    for fn in nc.m.functions:
        for blk in fn.blocks:
            new_insts = []
            last_sig = None
            for inst in blk.instructions:
                tn = type(inst).__name__
                if getattr(inst, "engine", None) != mybir.EngineType.PE:
                    new_insts.append(inst)
                    continue
                if tn == "InstLdweights":
                    ap = inst.ins[0]
                    sig = (
                        getattr(ap, "memref", None),
                        getattr(ap, "offset", None),
                        str(getattr(ap, "ap", None)),
                        getattr(ap, "dtype", None),
                    )
                    if sig == last_sig:
                        si = inst.sync_info
                        if si is not None and (si.on_wait or si.on_update):
                            nop = mybir.InstNoOp(
                                name=f"{inst.name}-ldwdup",
                                ins=[],
                                outs=[],
                                bass_is_fusable=False,
                            )
                            nop.engine = inst.engine
                            nop.sync_info = si
                            new_insts.append(nop)
                        continue
                    last_sig = sig
                elif tn != "InstMatmult":
                    last_sig = None
                new_insts.append(inst)
            blk.instructions[:] = new_insts


def _split_multi_waits(nc):
    """This walrus build accepts only one sem-wait per instruction; hoist
    extra waits onto same-engine NoOps inserted just before the owner."""
    import concourse.mybir as mybir

    for fn in nc.m.functions:
        for blk in fn.blocks:
            new_insts = []
            for inst in blk.instructions:
                si = inst.sync_info
                if si is not None and len(si.on_wait) > 1:
                    waits = list(si.on_wait)
                    si.on_wait = [waits[-1]]
                    for j, w in enumerate(waits[:-1]):
                        nop = mybir.InstNoOp(
                            name=f"{inst.name}-waitsplit-{j}",
                            ins=[],
                            outs=[],
                            bass_is_fusable=False,
                        )
                        nop.engine = inst.engine
                        nop.sync_info = mybir.SyncInfo(on_wait=[w], on_update=[])
                        new_insts.append(nop)
                new_insts.append(inst)
            blk.instructions[:] = new_insts


def _build_nc():
    import concourse.bass as bass
    import concourse.mybir as mybir
    from concourse.tile import TileContext
    from concourse.masks import make_identity

    _patch_tile_drain()

    fp32 = mybir.dt.float32
    bf16 = mybir.dt.bfloat16
    Exp = mybir.ActivationFunctionType.Exp
    ADD = mybir.AluOpType.add

    nc = bass.Bass()

    EC = E // 128  # 8 contraction chunks
    CW = 1024  # K/V chunk width (columns of xT)
    NCH = T // CW  # 4 chunks
    NS = T // 128  # 32 s-tiles
    NK = NS // 2  # 16 s-tile pairs per query block
    NJUNK = 14  # PE warm-up matmuls during the startup DMA wait

    # xT is staged host-side as 8 contiguous pieces [128, EC*512]
    xT_ext = nc.declare_dram_parameter(
        "xTp", [NCH * 2 * 128, EC * 512], bf16, isOutput=False
    )
    wqq_ext = nc.declare_dram_parameter("w_qq", [E, 128], bf16, isOutput=False)
    wvk_ext = nc.declare_dram_parameter("w_vk", [E, 128], bf16, isOutput=False)
    bqq_ext = nc.declare_dram_parameter("b_qq", [128, 1], fp32, isOutput=False)
    bvk_ext = nc.declare_dram_parameter("b_vk", [128, 1], fp32, isOutput=False)
    out_ext = nc.declare_dram_parameter("outT", [D + 1, HALF], fp32, isOutput=True)

    with TileContext(nc) as tc:
        with (
            tc.tile_pool(name="w", bufs=1) as wpool,
            tc.tile_pool(name="xt", bufs=2) as xtpool,
            tc.tile_pool(name="big", bufs=1) as bigpool,
            tc.tile_pool(name="pt", bufs=4) as ptpool,
            tc.tile_pool(name="oc", bufs=2) as ocpool,
            tc.tile_pool(name="ps", bufs=2, space="PSUM") as pspool,
            tc.tile_pool(name="pj", bufs=2, space="PSUM") as pjpool,
            tc.tile_pool(name="po", bufs=1, space="PSUM") as popool,
        ):
            # --- input DMAs: chunk-0 pieces + weights first, the rest
            # follow; every trigger is a plain linear copy ---
            xts = {}

            def emit_xt(cj, h):
                xt = xtpool.tile(
                    [128, EC * 512], bf16, tag="xt", name=f"xt{cj}_{h}"
                )
                idx = 2 * cj + h
                nc.sync.dma_start(
                    out=xt[:], in_=xT_ext[idx * 128 : (idx + 1) * 128, :]
                )
                xts[(cj, h)] = xt

            emit_xt(0, 0)
            wqq_sb = wpool.tile([128, EC * 128], bf16, tag="wqq")
            wvk_sb = wpool.tile([128, EC * 128], bf16, tag="wvk")
            nc.sync.dma_start(
                out=wqq_sb[:], in_=wqq_ext[:].rearrange("(c p) m -> p c m", p=128)
            )
            nc.sync.dma_start(
                out=wvk_sb[:], in_=wvk_ext[:].rearrange("(c p) m -> p c m", p=128)
            )
            emit_xt(0, 1)
            ball_sb = wpool.tile([128, 2], fp32, tag="ball")
            nc.sync.dma_start(out=ball_sb[:, 0:1], in_=bqq_ext[:])
            nc.sync.dma_start(out=ball_sb[:, 1:2], in_=bvk_ext[:])
            bqq_sb = ball_sb[:, 0:1]
            bvk_sb = ball_sb[:, 1:2]

            # --- PE warm-up during the DMA wait (HAM unthrottle) + exp
            # table preload on ScalarE ---
            jw_sb = wpool.tile([128, 64], bf16, tag="jw")
            nc.vector.memset(jw_sb[:], 0.0)
            tl1 = wpool.tile([128, 1], fp32, tag="tl1")
            nc.scalar.activation(tl1[:], jw_sb[:, 0:1], Exp)
            psj = pjpool.tile([128, 64], fp32, tag="pj", name="psjunk")
            for _ in range(NJUNK):
                nc.tensor.matmul(psj[0:64, 0:64], lhsT=jw_sb[:], rhs=jw_sb[:])

            qq_sb = bigpool.tile([128, HALF], bf16, tag="qq")
            vk_sb = bigpool.tile([128, T], bf16, tag="vk")
            ktlo_sb = bigpool.tile([64, T], bf16, tag="ktlo")
            # V' tiles: [128, 65] per s-tile, ones in column 64
            vp_sb = bigpool.tile([128, NS * 65], bf16, tag="vp")
            nc.vector.memset(vp_sb[:], 1.0)
            ident = wpool.tile([64, 64], bf16, tag="ident")
            make_identity(nc, ident[:])

            # --- projection pieces: per (chunk, q|vk, 512-col half):
            # 8 matmuls + bias (+ KT_lo copy and V' transposes for vk
            # halves); the scheduler back-fills these into PE slack ---
            def emit_half(cj, kind, h):
                pj = pjpool.tile(
                    [128, 512], fp32, tag="pj", name=f"pj{cj}{kind}{h}"
                )
                w_sb = wqq_sb if kind == "q" else wvk_sb
                xt = xts[("q", h)] if (kind == "q" and cj == 1) else xts[(cj, h)]
                for e in range(EC):
                    nc.tensor.matmul(
                        pj[:],
                        lhsT=w_sb[:, e * 128 : (e + 1) * 128],
                        rhs=xt[:, e * 512 : (e + 1) * 512],
                        start=(e == 0),
                        stop=(e == EC - 1),
                    )
                cols = slice(cj * CW + h * 512, cj * CW + h * 512 + 512)
                if kind == "q":
                    nc.vector.tensor_scalar(
                        qq_sb[:, cols], pj[:], bqq_sb[:], None, op0=ADD
                    )
                else:
                    nc.vector.tensor_scalar(
                        vk_sb[:, cols], pj[:], bvk_sb[:], None, op0=ADD
                    )
                    # KT to partitions 0:64 (cross-partition => DMA)
                    nc.sync.dma_start(
                        out=ktlo_sb[:, cols], in_=vk_sb[64:128, cols]
                    )
                    # V' build for this half's 4 s-tiles: PE transpose +
                    # DVE copy, medium priority (beat other proj work)
                    with tc.high_priority(offset=500_000):
                        for si in range(8 * cj + 4 * h, 8 * cj + 4 * h + 4):
                            pvt = pjpool.tile(
                                [128, 64], bf16, tag="pj", name=f"pvt{si}"
                            )
                            nc.tensor.transpose(
                                pvt[:],
                                vk_sb[0:64, si * 128 : (si + 1) * 128],
                                ident[:],
                            )
                            nc.vector.tensor_copy(
                                out=vp_sb[:, si * 65 : si * 65 + 64], in_=pvt[:]
                            )

            ps_out = {}

            def emit_scores(th, k):
                """Score matmuls for s-tile pair (2k, 2k+1) x query block th.
                Even tile contracts on PE rows 0:63, odd on 64:127 -> the
                pairs run concurrently in the array."""
                sA, sB = 2 * k, 2 * k + 1
                psa = pspool.tile([128, 1024], fp32, tag="ps", name=f"psa{th}_{k}")
                psb = pspool.tile([128, 1024], fp32, tag="ps", name=f"psb{th}_{k}")
                for half in range(2):
                    mc = slice(half * 512, (half + 1) * 512)
                    qcols = slice(
                        th * 1024 + half * 512, th * 1024 + half * 512 + 512
                    )
                    nc.tensor.matmul(
                        psa[:, mc],
                        lhsT=ktlo_sb[:, sA * 128 : (sA + 1) * 128],
                        rhs=qq_sb[0:64, qcols],
                    )
                    nc.tensor.matmul(
                        psb[:, mc],
                        lhsT=vk_sb[64:128, sB * 128 : (sB + 1) * 128],
                        rhs=qq_sb[64:128, qcols],
                    )
                return psa, psb

            def emit_expav(th, k, psa, psb):
                """exp + AV accumulation for step (th, k)."""
                if th not in ps_out:
                    ps_out[th] = popool.tile(
                        [D + 1, 1024], fp32, tag="po", name=f"pso{th}"
                    )
                po = ps_out[th]
                sA, sB = 2 * k, 2 * k + 1
                pta = ptpool.tile([128, 1024], bf16, tag="pt")
                ptb = ptpool.tile([128, 1024], bf16, tag="pt")
                nc.scalar.activation(pta[:], psa[:], Exp, scale=0.125)
                nc.scalar.activation(ptb[:], psb[:], Exp, scale=0.125)
                for si, pt in ((sA, pta), (sB, ptb)):
                    for half in range(2):
                        mc = slice(half * 512, (half + 1) * 512)
                        nc.tensor.matmul(
                            po[:, mc],
                            lhsT=vp_sb[:, si * 65 : (si + 1) * 65],
                            rhs=pt[:, mc],
                            start=(k == 0 and si == sA),
                            stop=(k == NK - 1 and si == sB),
                        )
                return pta

            def emit_flush(th):
                tcols = slice(th * 1024, (th + 1) * 1024)
                oc = ocpool.tile([D + 1, 1024], fp32, tag="oc")
                nc.vector.tensor_copy(out=oc[:], in_=ps_out[th][:])
                nc.sync.dma_start(out=out_ext[:, tcols], in_=oc[:])
                del ps_out[th]

            # --- projection in chunk order, interleaved with the next
            # chunk's loads: the 2-slot xt ring makes chunk c+1's DMA wait
            # for chunk c's matmuls, so the scheduler cannot pre-fill the
            # PE queue with future projection work -- each chunk becomes
            # ready in a wave just as the attention stream wants it ---
            for h in range(2):
                emit_half(0, "q", h)
                emit_half(0, "vk", h)
            for cj in range(1, NCH):
                for h in range(2):
                    emit_xt(cj, h)
                    emit_half(cj, "vk", h)
            # chunk 1's Q (only needed by th=1) reads its own late copy of
            # the chunk-1 pieces so the load ring above stays light
            for h in range(2):
                xq = xtpool.tile(
                    [128, EC * 512], bf16, tag="xq", name=f"xq1_{h}", bufs=2
                )
                idx = 2 + h
                nc.sync.dma_start(
                    out=xq[:], in_=xT_ext[idx * 128 : (idx + 1) * 128, :]
                )
                xts[("q", h)] = xq
                emit_half(1, "q", h)

            # --- attention stream at top priority: whenever its next
            # instruction is ready it wins the engine ---
            steps = [(0, k) for k in range(NK)] + [(1, k) for k in range(NK)]
            jnk2 = [None]
            pend = None
            for si, (th, k) in enumerate(steps):
                with tc.high_priority(offset=1_000_000):
                    psa, psb = emit_scores(th, k)
                    if pend is not None:
                        pta_prev = emit_expav(*pend)
                        if pend[0] == 0 and pend[1] == NK - 1:
                            emit_flush(0)
                    pend = (th, k, psa, psb)
                if th == 1 and k < NK - 2 and pend is not None:
                    # low-priority HAM keep-alive: th=1 leaves the PE ~30%
                    # idle which re-throttles it to 1.2 GHz; these fillers
                    # ride each step's idle (gated on that step's exp)
                    if jnk2[0] is None:
                        jnk2[0] = pjpool.tile(
                            [64, 512], fp32, tag="pj", name="jnk2", bufs=None
                        )
                    for _ in range(3):
                        nc.tensor.matmul(
                            jnk2[0][0:64, :], lhsT=jw_sb[:], rhs=pta_prev[:, 0:512]
                        )
            with tc.high_priority(offset=1_000_000):
                emit_expav(*pend)
                emit_flush(1)

    nc.finalize()
    _dedupe_ldweights(nc)
    _split_multi_waits(nc)
    return nc


def _get_nc():
    if "nc" not in _compiled:
        _compiled["nc"] = _build_nc()
    return _compiled["nc"]


def _make_in_maps(x, Wq, bq, Wk, bk, Wv, bv):
    import ml_dtypes

    bf16 = ml_dtypes.bfloat16
    w_qq = np.concatenate([Wq, Wq], axis=1).astype(bf16)  # [E, 128]
    w_vk = np.concatenate([Wv, Wk], axis=1).astype(bf16)  # [E, 128]
    b_qq = np.concatenate([bq, bq]).reshape(128, 1).astype(np.float32)
    b_vk = np.concatenate([bv, bk]).reshape(128, 1).astype(np.float32)

    xT = np.transpose(x, (0, 2, 1))  # [B, E, T]
    in_maps = []
    for c in range(NCORES):
        b, h = divmod(c, 2)
        xb = xT[b]
        if h == 1:
            # permute so the core's own query half is in columns 0:HALF
            xb = np.concatenate([xb[:, HALF:], xb[:, :HALF]], axis=1)
        # pre-chunk into 8 contiguous pieces [128p, 8e, 512t]
        # piece idx = 2*cj + hh covers columns idx*512:(idx+1)*512
        xp = np.ascontiguousarray(
            xb.reshape(8, 128, 8, 512).transpose(2, 1, 0, 3).reshape(1024, 4096)
        ).astype(bf16)
        in_maps.append(
            {"xTp": xp, "w_qq": w_qq, "w_vk": w_vk, "b_qq": b_qq, "b_vk": b_vk}
        )
    return in_maps


def _assemble(results):
    out = np.empty((B, T, D), np.float32)
    for c in range(NCORES):
        b, h = divmod(c, 2)
        ot = results[c]["outT"]  # [65, HALF]
        out[b, h * HALF : (h + 1) * HALF, :] = (ot[:D] / ot[D : D + 1]).T
    return out


def kernel(x, Wq, bq, Wk, bk, Wv, bv):
    x = np.asarray(x, dtype=np.float32)
    Wq = np.asarray(Wq, dtype=np.float32)
    Wk = np.asarray(Wk, dtype=np.float32)
    Wv = np.asarray(Wv, dtype=np.float32)
    bq = np.asarray(bq, dtype=np.float32)
    bk = np.asarray(bk, dtype=np.float32)
    bv = np.asarray(bv, dtype=np.float32)

    from concourse.bass_utils import run_bass_kernel_spmd

    in_maps = _make_in_maps(x, Wq, bq, Wk, bk, Wv, bv)
    nc = _get_nc()
    res = run_bass_kernel_spmd(nc, in_maps, list(range(NCORES)))
    return _assemble(res.results)
